# revision 1
# baseline (speedup 1.0000x reference)
"""LoRA BF16 Linear kernel for 8x Trainium2 NeuronCores.

Computes out = x @ W^T + b + 2.0 * (x @ A^T) @ B^T
  x [4,4096,4096] f32, W [4096,4096] f32, b [4096] f32, A [16,4096], B [4096,16]

Strategy: data-parallel over the 16384 tokens (2048 per core). Each core:
  - holds x_shard^T (bf16) resident in SBUF (in m-blocks)
  - streams W^T (bf16) in output-column slabs
  - computes out[tokens, dout] in PSUM via 32 K=128 bf16 matmuls
  - LoRA branch + bias are folded in as one extra augmented matmul per tile:
      rows 0..15 = (2A @ x^T) (computed on-device), row 16 = ones;
      W-side rows 0..15 = B^T, row 16 = b.
No collectives needed; host shards inputs and concatenates core outputs.
"""

import os
import numpy as np
import ml_dtypes
from contextlib import ExitStack

BF16 = ml_dtypes.bfloat16

# Problem shapes (hardcoded per harness contract)
B_, S, D_IN, D_OUT, R = 4, 4096, 4096, 4096, 16
N_CORES = 8
TOK = B_ * S                 # 16384 tokens total
T = TOK // N_CORES           # 2048 tokens per core
KO = D_IN // 128             # 32 k-tiles
SCALING = 32.0 / 16.0

_CACHE: dict = {}
_ONES = np.ones((1, T), dtype=BF16)

VARIANT = os.environ.get("KERNEL_VARIANT", "v1")


def _build_bass(variant=None):
    import concourse.bacc as bacc
    import concourse.mybir as mybir
    import concourse.tile as tile
    from concourse.bass import ts

    variant = variant or VARIANT
    flags = variant.split("-")
    base = flags[0]
    if base == "v0":
        NT, MB, k_stride = 256, 1, 1
    elif base == "v1":
        NT, MB, k_stride = 512, 2, 1
    elif base == "v2":  # PE-light probe: only 1 of 32 k matmuls (WRONG results)
        NT, MB, k_stride = 256, 1, 32
    else:
        raise ValueError(variant)
    NOSTORE = "nostore" in flags   # skip output DMA (wrong results)
    NOCOPY = "nocopy" in flags     # skip psum->sbuf copy too
    NOWT = "nowt" in flags         # load wt slab once, reuse (wrong results)
    PELIGHT = "pelight" in flags   # only 1 of 32 k matmuls
    if PELIGHT:
        k_stride = 32

    N_TILES = D_OUT // NT
    TB = T // MB            # tokens per block
    M_TILES = TB // 128     # m-tiles per block

    nc = bacc.Bacc("TRN2", target_bir_lowering=False, debug=False)
    BF = mybir.dt.bfloat16
    F32 = mybir.dt.float32

    xT = nc.dram_tensor("xT", [D_IN, T], BF, kind="ExternalInput")
    WT = nc.dram_tensor("WT", [D_IN, D_OUT], BF, kind="ExternalInput")
    ATp = nc.dram_tensor("ATp", [128, KO * R], BF, kind="ExternalInput")
    WBaug = nc.dram_tensor("WBaug", [128, D_OUT], BF, kind="ExternalInput")
    ones = nc.dram_tensor("ones", [1, T], BF, kind="ExternalInput")
    out = nc.dram_tensor("out", [T, D_OUT], F32, kind="ExternalOutput")

    xT_r = xT.ap().rearrange("(ko p) t -> p ko t", p=128)
    WT_r = WT.ap().rearrange("(ko p) o -> p ko o", p=128)
    out_ap = out.ap()

    with tile.TileContext(nc) as tc:
        with ExitStack() as ctx:
            resident = ctx.enter_context(tc.tile_pool(name="resident", bufs=1))
            xpool = ctx.enter_context(tc.tile_pool(name="xpool", bufs=1))
            wtpool = ctx.enter_context(tc.tile_pool(name="wtpool", bufs=2))
            opool = ctx.enter_context(tc.tile_pool(name="opool", bufs=8))
            pspool = ctx.enter_context(
                tc.tile_pool(name="pspool", bufs=7, space="PSUM")
            )

            AT_sb = resident.tile([128, KO * R], BF)
            nc.sync.dma_start(out=AT_sb, in_=ATp.ap())
            AT_r = AT_sb.rearrange("p (ko r) -> p ko r", r=R)
            WB_sb = resident.tile([128, D_OUT], BF)
            nc.sync.dma_start(out=WB_sb, in_=WBaug.ap())

            # xa^T augmented: rows 0-15 = 2*A@x^T, row 16 = ones, rest zero
            xaT_sb = resident.tile([128, T], BF)
            nc.any.memset(xaT_sb, 0.0)
            nc.sync.dma_start(out=xaT_sb[R : R + 1, :], in_=ones.ap())

            for mb in range(MB):
                # Resident x^T block: [128, 32, TB] bf16
                xT_sb = xpool.tile([128, KO, TB], BF, tag="xTblk")
                for ko in range(KO):
                    nc.sync.dma_start(
                        out=xT_sb[:, ko, :],
                        in_=xT_r[:, ko, ts(mb, TB)],
                    )

                # Prologue: xa^T[r, t] = sum_k (2A)^T[k, r] * x^T[k, t]
                for tw in range(TB // 512):
                    ps_xa = pspool.tile([16, 512], F32, tag="ps_xa", bufs=1)
                    for ko in range(KO):
                        nc.tensor.matmul(
                            ps_xa,
                            AT_r[:, ko, :],
                            xT_sb[:, ko, ts(tw, 512)],
                            start=(ko == 0),
                            stop=(ko == KO - 1),
                        )
                    nc.vector.tensor_copy(
                        out=xaT_sb[0:R, mb * TB + tw * 512 : mb * TB + (tw + 1) * 512],
                        in_=ps_xa,
                    )

                # Main: out[m-tile, n-tile] = sum_ko xT_k^T @ WT_k  (+ aug)
                wt_cached = None
                for n in range(N_TILES):
                    if NOWT and wt_cached is not None:
                        wt_sb = wt_cached
                    else:
                        wt_sb = wtpool.tile([128, KO, NT], BF, tag="wt")
                        for kh in range(2):
                            nc.sync.dma_start(
                                out=wt_sb[:, ts(kh, KO // 2), :],
                                in_=WT_r[:, ts(kh, KO // 2), ts(n, NT)],
                            )
                        wt_cached = wt_sb
                    for m in range(M_TILES):
                        ps = pspool.tile([128, NT], F32, tag="ps")
                        for ko in range(0, KO, k_stride):
                            nc.tensor.matmul(
                                ps,
                                xT_sb[:, ko, ts(m, 128)],
                                wt_sb[:, ko, :],
                                start=(ko == 0),
                                stop=False,
                            )
                        gm = mb * M_TILES + m  # global m-tile
                        nc.tensor.matmul(
                            ps,
                            xaT_sb[:, ts(gm, 128)],
                            WB_sb[:, ts(n, NT)],
                            start=False,
                            stop=True,
                        )
                        if not NOCOPY:
                            ob = opool.tile([128, NT], F32, tag="ob")
                            nc.vector.tensor_copy(out=ob, in_=ps)
                            if not NOSTORE:
                                nc.scalar.dma_start(
                                    out=out_ap[ts(gm, 128), ts(n, NT)], in_=ob
                                )

    nc.compile()
    return nc


def _get_nc(variant=None):
    key = "nc_" + (variant or VARIANT)
    if key not in _CACHE:
        _CACHE[key] = _build_bass(variant)
    return _CACHE[key]


def _prep_inputs(x, W, b, A, B):
    xf = np.ascontiguousarray(x.reshape(TOK, D_IN)).astype(BF16)
    WTh = np.ascontiguousarray(W.T).astype(BF16)            # [d_in, d_out]
    ATh = (SCALING * A).T.astype(BF16)                       # [d_in, r]
    ATp = np.ascontiguousarray(
        ATh.reshape(KO, 128, R).transpose(1, 0, 2).reshape(128, KO * R)
    )
    WBh = np.zeros((128, D_OUT), dtype=BF16)
    WBh[0:R] = B.T.astype(BF16)
    WBh[R] = b.astype(BF16)

    in_maps = []
    for c in range(N_CORES):
        xTc = np.ascontiguousarray(xf[c * T : (c + 1) * T].T)  # [d_in, T] bf16
        in_maps.append(
            {"xT": xTc, "WT": WTh, "ATp": ATp, "WBaug": WBh, "ones": _ONES}
        )
    return in_maps


def kernel(x, W, b, A, B):
    from concourse.bass_utils import run_bass_kernel_spmd

    nc = _get_nc()
    in_maps = _prep_inputs(x, W, b, A, B)
    res = run_bass_kernel_spmd(nc, in_maps, core_ids=list(range(N_CORES)))
    outs = [r["out"] for r in res.results]
    return np.concatenate(outs, axis=0).reshape(B_, S, D_OUT).astype(np.float32)



# revision 12
# speedup vs baseline: 6.2809x; 6.2809x over previous
"""LoRA BF16 Linear kernel for 8x Trainium2 NeuronCores.

Computes out = x @ W^T + b + 2.0 * (x @ A^T) @ B^T
  x [4,4096,4096] f32, W [4096,4096] f32, b [4096] f32, A [16,4096], B [4096,16]

Strategy: data-parallel over the 16384 tokens (2048 per core).
The LoRA rank-16 update is folded into W on the host (W' = W + 2*B@A,
a 0.5 GFLOP host-side rank-16 update), so the device runs a pure GEMM:
  - each core holds its x_shard^T (bf16) resident in SBUF (in m-blocks)
  - streams W'^T (bf16) in 512-wide output-column slabs
  - accumulates out[128 tokens, 512 outs] tiles in PSUM via 32 K=128
    bf16 matmuls
  - bias is added during the PSUM->SBUF copy on the DVE engine
    (scalar_tensor_tensor), so the PE does no extra bias/LoRA work.
No collectives needed; host shards inputs and concatenates core outputs.
"""

import os
import numpy as np
import ml_dtypes
from contextlib import ExitStack

BF16 = ml_dtypes.bfloat16

# Problem shapes (hardcoded per harness contract)
B_, S, D_IN, D_OUT, R = 4, 4096, 4096, 4096, 16
N_CORES = 8
TOK = B_ * S                 # 16384 tokens total
T = TOK // N_CORES           # 2048 tokens per core
KO = D_IN // 128             # 32 k-tiles
SCALING = 32.0 / 16.0

_CACHE: dict = {}

VARIANT = os.environ.get("KERNEL_VARIANT", "v4")


def _build_bass(variant=None):
    import concourse.bacc as bacc
    import concourse.mybir as mybir
    import concourse.tile as tile
    from concourse.bass import ts

    variant = variant or VARIANT
    flags = variant.split("-")
    base = flags[0]
    assert base in ("v3", "v4"), variant
    NT, MB = 512, 2
    REPS = 1
    for f in flags:
        if f.startswith("rep"):
            REPS = int(f[3:])

    N_TILES = D_OUT // NT
    TB = T // MB            # tokens per block
    M_TILES = TB // 128     # m-tiles per block

    nc = bacc.Bacc("TRN2", target_bir_lowering=False, debug=False)
    BF = mybir.dt.bfloat16
    F32 = mybir.dt.float32

    xT = nc.dram_tensor("xT", [D_IN, T], BF, kind="ExternalInput")
    WT = nc.dram_tensor("WT", [D_IN, D_OUT], BF, kind="ExternalInput")
    bias = nc.dram_tensor(
        "bias", [128, D_OUT], F32 if base == "v3" else BF, kind="ExternalInput"
    )
    out = nc.dram_tensor("out", [T, D_OUT], F32, kind="ExternalOutput")

    xT_r = xT.ap().rearrange("(ko p) t -> p ko t", p=128)
    WT_r = WT.ap().rearrange("(ko p) o -> p ko o", p=128)
    out_ap = out.ap()

    if base == "v3":
        with tile.TileContext(nc) as tc:
            with ExitStack() as ctx:
                resident = ctx.enter_context(tc.tile_pool(name="resident", bufs=1))
                xpool = ctx.enter_context(tc.tile_pool(name="xpool", bufs=1))
                wtpool = ctx.enter_context(tc.tile_pool(name="wtpool", bufs=2))
                opool = ctx.enter_context(tc.tile_pool(name="opool", bufs=8))
                pspool = ctx.enter_context(
                    tc.tile_pool(name="pspool", bufs=8, space="PSUM")
                )

                bias_sb = resident.tile([128, D_OUT], F32)
                nc.sync.dma_start(out=bias_sb, in_=bias.ap())

                for rep in range(REPS):
                    for mb in range(MB):
                        # Resident x^T block: [128, 32, TB] bf16
                        xT_sb = xpool.tile([128, KO, TB], BF, tag="xTblk")
                        for ko in range(KO):
                            nc.sync.dma_start(
                                out=xT_sb[:, ko, :],
                                in_=xT_r[:, ko, ts(mb, TB)],
                            )

                        # out[m, n] = sum_ko xT_k^T @ WT_k ; +bias on DVE
                        for n in range(N_TILES):
                            wt_sb = wtpool.tile([128, KO, NT], BF, tag="wt")
                            for kh in range(2):
                                nc.sync.dma_start(
                                    out=wt_sb[:, ts(kh, KO // 2), :],
                                    in_=WT_r[:, ts(kh, KO // 2), ts(n, NT)],
                                )
                            for m in range(M_TILES):
                                ps = pspool.tile([128, NT], F32, tag="ps")
                                for ko in range(KO):
                                    nc.tensor.matmul(
                                        ps,
                                        xT_sb[:, ko, ts(m, 128)],
                                        wt_sb[:, ko, :],
                                        start=(ko == 0),
                                        stop=(ko == KO - 1),
                                    )
                                gm = mb * M_TILES + m  # global m-tile
                                ob = opool.tile([128, NT], F32, tag="ob")
                                nc.vector.scalar_tensor_tensor(
                                    out=ob,
                                    in0=ps,
                                    scalar=1.0,
                                    in1=bias_sb[:, ts(n, NT)],
                                    op0=mybir.AluOpType.mult,
                                    op1=mybir.AluOpType.add,
                                )
                                nc.scalar.dma_start(
                                    out=out_ap[ts(gm, 128), ts(n, NT)], in_=ob
                                )
        nc.compile()
        return nc

    # v4: x double-buffered (bufs=2) so the mb=1 block prefetches during
    # mb=0 compute; x DMAs on the Pool queue (wt owns SP, outputs own
    # Activation); ko-outer matmul order on the very first slab so the PE
    # consumes x k-tiles in DMA-delivery order instead of head-of-line
    # blocking on m0's full K; bf16 bias + opool bufs=4 to fit SBUF.
    with tile.TileContext(nc) as tc:
        with ExitStack() as ctx:
            resident = ctx.enter_context(tc.tile_pool(name="resident", bufs=1))
            xpool = ctx.enter_context(tc.tile_pool(name="xpool", bufs=2))
            wtpool = ctx.enter_context(tc.tile_pool(name="wtpool", bufs=2))
            opool = ctx.enter_context(tc.tile_pool(name="opool", bufs=3))
            pspool = ctx.enter_context(
                tc.tile_pool(name="pspool", bufs=8, space="PSUM")
            )

            bias_sb = resident.tile([128, D_OUT], BF)
            nc.scalar.dma_start(out=bias_sb, in_=bias.ap())

            for rep in range(REPS):
                for mb in range(MB):
                    xT_sb = xpool.tile([128, KO, TB], BF, tag="xTblk")
                    for ko in range(KO):
                        nc.gpsimd.dma_start(
                            out=xT_sb[:, ko, :],
                            in_=xT_r[:, ko, ts(mb, TB)],
                        )

                    for n in range(N_TILES):
                        wt_sb = wtpool.tile([128, KO, NT], BF, tag="wt")
                        for kh in range(4):
                            nc.sync.dma_start(
                                out=wt_sb[:, ts(kh, KO // 4), :],
                                in_=WT_r[:, ts(kh, KO // 4), ts(n, NT)],
                            )

                        first_slab = rep == 0 and mb == 0 and n == 0
                        if first_slab:
                            # ko-outer: 8 concurrent PSUM groups, consume
                            # each x k-tile as it lands
                            pss = []
                            for _pi in range(M_TILES):
                                ps0 = pspool.tile(
                                    [128, NT], F32, tag="ps", name=f"ps0_{_pi}"
                                )
                                pss.append(ps0)
                            for ko in range(KO):
                                for m in range(M_TILES):
                                    nc.tensor.matmul(
                                        pss[m],
                                        xT_sb[:, ko, ts(m, 128)],
                                        wt_sb[:, ko, :],
                                        start=(ko == 0),
                                        stop=(ko == KO - 1),
                                    )
                            for m in range(M_TILES):
                                gm = mb * M_TILES + m
                                ob = opool.tile([128, NT], F32, tag="ob")
                                nc.vector.scalar_tensor_tensor(
                                    out=ob,
                                    in0=pss[m],
                                    scalar=1.0,
                                    in1=bias_sb[:, ts(n, NT)],
                                    op0=mybir.AluOpType.mult,
                                    op1=mybir.AluOpType.add,
                                )
                                nc.scalar.dma_start(
                                    out=out_ap[ts(gm, 128), ts(n, NT)], in_=ob
                                )
                            continue

                        for m in range(M_TILES):
                            ps = pspool.tile([128, NT], F32, tag="ps")
                            for ko in range(KO):
                                nc.tensor.matmul(
                                    ps,
                                    xT_sb[:, ko, ts(m, 128)],
                                    wt_sb[:, ko, :],
                                    start=(ko == 0),
                                    stop=(ko == KO - 1),
                                )
                            gm = mb * M_TILES + m
                            ob = opool.tile([128, NT], F32, tag="ob")
                            nc.vector.scalar_tensor_tensor(
                                out=ob,
                                in0=ps,
                                scalar=1.0,
                                in1=bias_sb[:, ts(n, NT)],
                                op0=mybir.AluOpType.mult,
                                op1=mybir.AluOpType.add,
                            )
                            nc.scalar.dma_start(
                                out=out_ap[ts(gm, 128), ts(n, NT)], in_=ob
                            )

    nc.compile()
    return nc


def _get_nc(variant=None):
    key = "nc_" + (variant or VARIANT)
    if key not in _CACHE:
        _CACHE[key] = _build_bass(variant)
    return _CACHE[key]


def _prep_inputs(x, W, b, A, B, variant=None):
    variant = variant or VARIANT
    base = variant.split("-")[0]
    # Fold the LoRA rank-16 update into W on the host:
    #   out = x@W^T + b + 2*(x@A^T)@B^T = x@(W + 2*B@A)^T + b
    W2 = W.astype(np.float32) + SCALING * (
        B.astype(np.float32) @ A.astype(np.float32)
    )
    WTh = np.ascontiguousarray(W2.T).astype(BF16)            # [d_in, d_out]
    bias_dt = np.float32 if base == "v3" else BF16
    bias128 = np.broadcast_to(
        b.astype(bias_dt), (128, D_OUT)
    ).copy()                                                 # [128, d_out]

    xf = np.ascontiguousarray(x.reshape(TOK, D_IN)).astype(BF16)
    in_maps = []
    for c in range(N_CORES):
        xTc = np.ascontiguousarray(xf[c * T : (c + 1) * T].T)  # [d_in, T] bf16
        in_maps.append({"xT": xTc, "WT": WTh, "bias": bias128})
    return in_maps


def kernel(x, W, b, A, B):
    from concourse.bass_utils import run_bass_kernel_spmd

    nc = _get_nc()
    in_maps = _prep_inputs(x, W, b, A, B)
    res = run_bass_kernel_spmd(nc, in_maps, core_ids=list(range(N_CORES)))
    outs = [r["out"] for r in res.results]
    return np.concatenate(outs, axis=0).reshape(B_, S, D_OUT).astype(np.float32)
